# revision 1
# baseline (speedup 1.0000x reference)
"""Trainium2 Bass kernel: multi-head attention with quantum (cumprod-of-cos) transform.

Full-input contract: kernel(**inputs) takes the unsharded inputs and returns the
full [B, S, E] output. Internally shards over 8 NeuronCores: data-parallel over
batch (B=2) x tensor-parallel over head-groups (4 heads per core).

Per-core math (b = batch, g = head-group of 4 heads, EG = 256 e-dims):
  thetaT_{q,k} [EG, S] = W_slice @ x_b^T          (transposed layout, d on partitions)
  z = cumprod(cos(theta)) along d:
      q,k: log-space: exp(cumsum(ln(cos^2)/2)) * parity-sign(cumsum(cos<0))
           cumsum via block-diag upper-triangular matmul (one K=128 MM covers 2 heads)
      v:   direct layout [S, EG], Hillis-Steele shift-multiply cumprod along free axis
  scoresT [t, s] per head/t-tile = k_z-tile (stationary) x q_z (moving), K=d=64
  expS = exp(scoresT / 8)  (fused ACT, PSUM->SBUF; |scores|<=64 so exp<=e^8: no
         max-subtraction needed for fp32 safety)
  out-matmul: stationary [v_z | ones] -> acc rows 0:64 = unnormalized out^T,
         rows 64:128 = softmax denominator (replicated 64x) -- free denominator
  normalize: r = reciprocal_approx_fast(denom) on DVE; outz = acc[0:64] * r
  yT_partial [E, S] = WcT_slice (stationary) x outz    -- host sums 4 partials/batch

All matmul operands are float32r (full fp32 bits; PE runs them single-pass at
1 cyc/row for N>=256 vs 4 cyc/row for plain fp32). ACT table sets are kept to
two loads by batching all Sin ops before all Ln/Exp ops.
"""

import os
import sys

import numpy as np

if "/opt/trn_rl_repo" not in sys.path:
    sys.path.insert(0, "/opt/trn_rl_repo")

import concourse.bass as bass  # noqa: F401
import concourse.tile as tile
from concourse import bacc
from concourse import mybir
from concourse.bass_utils import run_bass_kernel_spmd

AF = mybir.ActivationFunctionType
ALU = mybir.AluOpType
F32 = mybir.dt.float32
F32R = mybir.dt.float32r
BF16 = mybir.dt.bfloat16
I16 = mybir.dt.int16

B, S, E, H, D = 2, 2048, 1024, 16, 64
NCORES = 8
HG = 4          # heads per core
EG = HG * D     # 256
P = 128
NT = S // P     # 16 t-tiles
KC = E // P     # 8 contraction tiles for the projections
HALF_PI = float(np.pi / 2)
INV_SQRT_D = 0.125  # 1/sqrt(64)

_PHASES = 4     # ablation switch for TimelineSim profiling
_SKIP = set()   # {'vproj','hs','qkproj'} ablations
_DEBUG = False   # add intermediate dumps


def _attention_and_final(tc, ozp, z_tiles, vzts, wc_t, yT):
    nc = tc.nc
    with (
        tc.tile_pool(name="psB", bufs=1, space="PSUM") as psB,
        tc.tile_pool(name="exps", bufs=3) as exq,
        tc.tile_pool(name="norm", bufs=2) as nrm,
    ):
        for h in range(HG):
            m = h // 2
            dbase = (h % 2) * D
            qz = z_tiles[("q", m)]
            kz = z_tiles[("k", m)]
            if (h % 2) == 0:
                oz = ozp.tile([P, S], F32R, tag=f"oz{m}", name=f"oz{m}")
                z_tiles[("oz", m)] = oz
            else:
                oz = z_tiles[("oz", m)]
            for sb in range(2):
                ssl0 = sb * 1024
                acc = psB.tile([P, 1024], F32, tag="acc", bufs=2,
                               name=f"acc{h}_{sb}")
                for t in range(NT):
                    sc = psB.tile([P, 1024], F32, tag="s", bufs=2,
                                  name=f"sc{h}_{t}{sb}")
                    for ch in range(2):
                        ssl = slice(ssl0 + ch * 512, ssl0 + (ch + 1) * 512)
                        nc.tensor.matmul(
                            sc[:, ch * 512:(ch + 1) * 512],
                            lhsT=kz[dbase:dbase + D, t * P:(t + 1) * P],
                            rhs=qz[dbase:dbase + D, ssl],
                            start=True, stop=True,
                        )
                    ex = exq.tile([P, 1024], F32R, tag="ex", name=f"ex{h}_{t}{sb}")
                    nc.scalar.activation(ex[:], sc[:], AF.Exp, scale=INV_SQRT_D)
                    for ch in range(2):
                        nc.tensor.matmul(
                            acc[:, ch * 512:(ch + 1) * 512],
                            lhsT=vzts[t][:, 2 * h:2 * h + 2, :].rearrange(
                                "p a d -> p (a d)"),
                            rhs=ex[:, ch * 512:(ch + 1) * 512],
                            start=(t == 0), stop=(t == NT - 1),
                        )
                lnd = nrm.tile([D, 1024], F32, tag="lnd", name=f"lnd{h}{sb}")
                nc.scalar.activation(lnd[:], acc[D:2 * D, :], AF.Ln)
                rec = nrm.tile([D, 1024], F32, tag="rec", name=f"rec{h}{sb}")
                nc.scalar.activation(rec[:], lnd[:], AF.Exp, scale=-1.0)
                nc.vector.tensor_tensor(
                    out=oz[dbase:dbase + D, ssl0:ssl0 + 1024],
                    in0=acc[0:D, :], in1=rec[:], op=ALU.mult,
                )

        # ---------------- Block 4: final projection ----------------
        with tc.tile_pool(name="y", bufs=3) as yp:
            for mo in range(E // P):
                for sb in range(2):
                    py = psB.tile([P, 1024], F32, tag="s", bufs=2,
                                  name=f"py{mo}{sb}")
                    for ch in range(2):
                        ssl = slice(sb * 1024 + ch * 512, sb * 1024 + (ch + 1) * 512)
                        for kk in range(2):
                            nc.tensor.matmul(
                                py[:, ch * 512:(ch + 1) * 512],
                                lhsT=wc_t[:, kk, mo * P:(mo + 1) * P],
                                rhs=z_tiles[("oz", kk)][:, ssl],
                                start=(kk == 0), stop=(kk == 1),
                            )
                    yt = yp.tile([P, 1024], F32, tag="y", name=f"yt{mo}{sb}")
                    nc.vector.tensor_copy(out=yt[:], in_=py[:])
                    nc.sync.dma_start(
                        out=yT[mo * P:(mo + 1) * P, sb * 1024:(sb + 1) * 1024],
                        in_=yt[:],
                    )


def _build_body(tc, xT, wqT, wkT, wvT, wcT, u128, u128f, yT, dbg_handles=None):
    nc = tc.nc

    with (
        tc.tile_pool(name="const", bufs=1) as const,
        tc.tile_pool(name="wc", bufs=1) as wcp,
        tc.tile_pool(name="vz", bufs=1) as vzp,
        tc.tile_pool(name="ltile", bufs=1) as lp,
        tc.tile_pool(name="nbt", bufs=1) as nbp,
    ):
        hp = const.tile([P, 1], F32)
        nc.vector.memset(hp[:], HALF_PI)
        u_t = const.tile([P, P], F32R)
        nc.sync.dma_start(out=u_t[:], in_=u128[:])
        u_f = const.tile([P, P], F32)
        nc.sync.dma_start(out=u_f[:], in_=u128f[:])
        u_bf = const.tile([P, P], BF16)
        nc.vector.tensor_copy(out=u_bf[:], in_=u_f[:])
        ones64 = const.tile([P, 1, D], F32)
        nc.vector.memset(ones64[:], 1.0)
        wc_t = wcp.tile([P, 2, E], F32R)
        nc.sync.dma_start(out=wc_t[:], in_=wcT.rearrange("(k p) e -> p k e", p=P))

        vzts = [
            vzp.tile([P, 8, D], F32R, tag=f"vz{t}", name=f"vz{t}") for t in range(NT)
        ]
        l_tiles = {}
        nb_tiles = {}
        z_tiles = {}

        # ------------ Block 1: x + {Wq,Wk,Wv} resident ------------
        with (
            tc.tile_pool(name="psA", bufs=4, space="PSUM") as psA,
            tc.tile_pool(name="x", bufs=KC) as xp,
            tc.tile_pool(name="wqkv", bufs=1) as wp,
            tc.tile_pool(name="hs", bufs=2) as hsp,
            tc.tile_pool(name="ctile", bufs=2) as cp,
        ):
            # weights first: the first v-proj matmul needs wv + x0 only
            wv_t = wp.tile([P, KC, EG], F32R, tag="wv")
            nc.sync.dma_start(
                out=wv_t[:], in_=wvT.rearrange("(k p) e -> p k e", p=P))
            wq_t = wp.tile([P, KC, EG], F32R, tag="wq")
            nc.sync.dma_start(
                out=wq_t[:], in_=wqT.rearrange("(k p) e -> p k e", p=P))
            wk_t = wp.tile([P, KC, EG], F32R, tag="wk")
            nc.sync.dma_start(
                out=wk_t[:], in_=wkT.rearrange("(k p) e -> p k e", p=P))
            xts = []
            for k in range(KC):
                xt = xp.tile([P, S], F32R, tag="x", name=f"x{k}")
                nc.sync.dma_start(out=xt[:], in_=xT[k * P:(k + 1) * P, :])
                xts.append(xt)

            # ---- v: direct layout [s, e], Hillis-Steele cumprod (free axis)
            for t in (range(NT) if 'vproj' not in _SKIP else []):
                pv = psA.tile([P, EG], F32, tag="s", name=f"pv{t}")
                for k in range(KC):
                    nc.tensor.matmul(
                        pv[:],
                        lhsT=xts[k][:, t * P:(t + 1) * P],
                        rhs=wv_t[:, k, :],
                        start=(k == 0),
                        stop=(k == KC - 1),
                    )
                va = hsp.tile([P, HG, D], F32, tag="va", name=f"va{t}")
                vb = hsp.tile([P, HG, D], F32, tag="vb", name=f"vb{t}")
                nc.scalar.activation(
                    va[:].rearrange("p h d -> p (h d)"), pv[:],
                    AF.Sin, bias=hp[:],
                )
                cur, other = va, vb
                stages = [1, 2, 4, 8, 16, 32] if 'hs' not in _SKIP else []
                for si, off in enumerate(stages):
                    if si == 5:
                        mul_out = vzts[t][:, 0:8:2, off:D]
                        cpy_out = vzts[t][:, 0:8:2, 0:off]
                    else:
                        mul_out = other[:, :, off:D]
                        cpy_out = other[:, :, 0:off]
                    nc.vector.tensor_tensor(
                        out=mul_out,
                        in0=cur[:, :, off:D],
                        in1=cur[:, :, 0:D - off],
                        op=ALU.mult,
                    )
                    nc.vector.tensor_copy(out=cpy_out, in_=cur[:, :, 0:off])
                    cur, other = other, cur
                nc.vector.tensor_copy(
                    out=vzts[t][:, 1:8:2, :],
                    in_=ones64[:].broadcast_to([P, HG, D]))

            # ---- q, k: theta^T [e, s] -> Sin -> {nb, sq stored in l-tile}
            qk = (("q", wq_t), ("k", wk_t)) if 'qkproj' not in _SKIP else ()
            for name, w_t in qk:
                for m in range(2):
                    c_t = cp.tile([P, S], F32, tag="c", name=f"c{name}{m}")
                    for sb in range(2):
                        th = psA.tile([P, 1024], F32, tag="s",
                                      name=f"th{name}{m}{sb}")
                        for ch in range(2):
                            for k in range(KC):
                                nc.tensor.matmul(
                                    th[:, ch * 512:(ch + 1) * 512],
                                    lhsT=w_t[:, k, m * P:(m + 1) * P],
                                    rhs=xts[k][:, sb * 1024 + ch * 512:
                                               sb * 1024 + (ch + 1) * 512],
                                    start=(k == 0),
                                    stop=(k == KC - 1),
                                )
                        nc.scalar.activation(
                            c_t[:, sb * 1024:(sb + 1) * 1024], th[:],
                            AF.Sin, bias=hp[:],
                        )
                    nb = nbp.tile([P, S], BF16, tag=f"nb_{name}{m}")
                    nc.vector.tensor_scalar(
                        out=nb[:], in0=c_t[:], scalar1=0.0, scalar2=None,
                        op0=ALU.is_lt,
                    )
                    nb_tiles[(name, m)] = nb
                    l_t = lp.tile([P, S], F32R, tag=f"l_{name}{m}")
                    for sb in range(2):
                        sqp = psA.tile([P, 1024], F32, tag="s",
                                       name=f"sq{name}{m}{sb}")
                        nc.vector.tensor_tensor(
                            out=sqp[:],
                            in0=c_t[:, sb * 1024:(sb + 1) * 1024],
                            in1=c_t[:, sb * 1024:(sb + 1) * 1024],
                            op=ALU.mult,
                        )
                        nc.scalar.activation(
                            l_t[:, sb * 1024:(sb + 1) * 1024], sqp[:], AF.Ln)
                    l_tiles[(name, m)] = l_t

        if _PHASES < 2:
            return
        # ------------ Block 2: cumsums at FD=2048, int16 parity ------------
        with (
            tc.tile_pool(name="ps2", bufs=1, space="PSUM") as ps2,
            tc.tile_pool(name="qwork", bufs=2) as qw,
        ):
            for name in ("q", "k"):
                for m in range(2):
                    l_t = l_tiles[(name, m)]
                    nb = nb_tiles[(name, m)]
                    z_t = l_t  # overwrite l with z (WAR tracked by Tile)
                    pl = ps2.tile([P, S], F32, tag="pl", name=f"pl{name}{m}")
                    pn = ps2.tile([P, S], F32, tag="pn", name=f"pn{name}{m}")
                    for ch in range(4):
                        sl = slice(ch * 512, (ch + 1) * 512)
                        nc.tensor.matmul(
                            pl[:, sl], lhsT=u_t[:], rhs=l_t[:, sl],
                            start=True, stop=True,
                        )
                        nc.tensor.matmul(
                            pn[:, sl], lhsT=u_bf[:], rhs=nb[:, sl],
                            start=True, stop=True,
                        )
                    mag = qw.tile([P, S], F32, tag="mag", name=f"mag{name}{m}")
                    nc.scalar.activation(mag[:], pl[:], AF.Exp, scale=0.5)
                    pari = qw.tile([P, S], I16, tag="pari", name=f"pi{name}{m}")
                    nc.vector.tensor_copy(out=pari[:], in_=pn[:])
                    nc.vector.tensor_scalar(
                        out=pari[:], in0=pari[:], scalar1=1, scalar2=None,
                        op0=ALU.bitwise_and,
                    )
                    sgn = qw.tile([P, S], F32, tag="sgn", name=f"sg{name}{m}")
                    nc.vector.tensor_scalar(
                        out=sgn[:], in0=pari[:], scalar1=-2.0, scalar2=1.0,
                        op0=ALU.mult, op1=ALU.add,
                    )
                    nc.vector.tensor_tensor(
                        out=z_t[:], in0=mag[:], in1=sgn[:], op=ALU.mult)
                    z_tiles[(name, m)] = z_t

        if _PHASES < 3:
            return
        # ------------ Blocks 3+4 ------------
        with tc.tile_pool(name="outz", bufs=1) as ozp:
            _attention_and_final(tc, ozp, z_tiles, vzts, wc_t, yT)
            if _DEBUG:
                dbg = {"dbg_zq0": z_tiles[("q", 0)], "dbg_zq1": z_tiles[("q", 1)],
                       "dbg_zk0": z_tiles[("k", 0)], "dbg_zk1": z_tiles[("k", 1)],
                       "dbg_oz0": z_tiles[("oz", 0)], "dbg_oz1": z_tiles[("oz", 1)],
                       "dbg_vz0": vzts[0].rearrange("p a d -> p (a d)"),
                       "dbg_vz7": vzts[7].rearrange("p a d -> p (a d)")}
                for nm, ap in dbg.items():
                    tc.nc.sync.dma_start(
                        out=dbg_handles[nm][:],
                        in_=ap.bitcast(F32) if ap.dtype != F32 else ap)


def build_bass():
    nc = bacc.Bacc(None, target_bir_lowering=False)
    xT = nc.dram_tensor("xT", [E, S], F32R, kind="ExternalInput")
    wqT = nc.dram_tensor("wqT", [E, EG], F32R, kind="ExternalInput")
    wkT = nc.dram_tensor("wkT", [E, EG], F32R, kind="ExternalInput")
    wvT = nc.dram_tensor("wvT", [E, EG], F32R, kind="ExternalInput")
    wcT = nc.dram_tensor("wcT", [EG, E], F32R, kind="ExternalInput")
    u128 = nc.dram_tensor("u128", [P, P], F32R, kind="ExternalInput")
    u128f = nc.dram_tensor("u128f", [P, P], F32, kind="ExternalInput")
    yT = nc.dram_tensor("yT", [E, S], F32, kind="ExternalOutput")
    dbg_handles = {}
    if _DEBUG:
        for nm, shp in (("dbg_zq0", [P, S]), ("dbg_zq1", [P, S]),
                        ("dbg_zk0", [P, S]), ("dbg_zk1", [P, S]),
                        ("dbg_oz0", [P, S]), ("dbg_oz1", [P, S]),
                        ("dbg_vz0", [P, 8 * D]), ("dbg_vz7", [P, 8 * D])):
            dbg_handles[nm] = nc.dram_tensor(nm, shp, F32, kind="ExternalOutput")
    with tile.TileContext(nc) as tc:
        _build_body(tc, xT[:], wqT[:], wkT[:], wvT[:], wcT[:], u128[:],
                    u128f[:], yT[:], dbg_handles)
    nc.finalize()
    return nc


_NC_CACHE = None


def _get_nc():
    global _NC_CACHE
    if _NC_CACHE is None:
        _NC_CACHE = build_bass()
    return _NC_CACHE


def _u128_host():
    i = np.arange(P)
    u = ((i[:, None] // D == i[None, :] // D) & (i[:, None] % D <= i[None, :] % D))
    return u.astype(np.float32)


def kernel(x, Wq, Wk, Wv, Wc, bc, **kw):
    x = np.asarray(x, np.float32)
    u128 = _u128_host()
    in_maps = []
    for c in range(NCORES):
        b, g = divmod(c, NCORES // B)
        sl = slice(g * EG, (g + 1) * EG)
        in_maps.append({
            "xT": np.ascontiguousarray(np.asarray(x[b]).T),
            "wqT": np.ascontiguousarray(np.asarray(Wq)[sl, :].T),
            "wkT": np.ascontiguousarray(np.asarray(Wk)[sl, :].T),
            "wvT": np.ascontiguousarray(np.asarray(Wv)[sl, :].T),
            "wcT": np.ascontiguousarray(np.asarray(Wc)[:, sl].T),
            "u128": u128,
            "u128f": u128,
        })
    nc = _get_nc()
    res = run_bass_kernel_spmd(
        nc, in_maps, core_ids=list(range(NCORES)),
        trace=bool(int(os.environ.get("QK_TRACE", "0"))),
    )
    y = np.zeros((B, S, E), np.float32)
    for c in range(NCORES):
        b = c // (NCORES // B)
        y[b] += res.results[c]["yT"].T
    y += np.asarray(bc, np.float32)
    globals()["_LAST_RESULT"] = res
    return y



# revision 2
# speedup vs baseline: 1.2875x; 1.2875x over previous
"""Trainium2 Bass kernel: multi-head attention with quantum (cumprod-of-cos) transform.

Full-input contract: kernel(**inputs) takes the unsharded inputs and returns the
full [B, S, E] output. Internally shards over 8 NeuronCores: data-parallel over
batch (B=2) x tensor-parallel over head-groups (4 heads per core).

Per-core pipeline (b = batch, g = head-group of 4 heads, EG = 256 e-dims):

Phase A (per 128-row s-tile, 16 tiles):
  theta [s, 768] = x_tile (stationary) @ [Wq|Wk|Wv] slices (moving, fp16)
  c = sin(theta + pi/2) = cos(theta)          (one ACT op per tile, fp16 out)
  z = cumprod(c) along d (64 per head) via tensor_tensor_scan (op0=mult,
      op1=bypass): q,k scans on DVE, v scans on GPSIMD(Pool)
  zq/zk tiles [s, d] are PE-transposed ([128,128] fp16, identity-matmul) to
      zqT/zkT [d, s]; psum->sbuf copies ride Pool
  vz tiles [t, 8, 64] interleave z_v with ones columns (denominator trick)

Phase B attention (per head, per 512-wide s-chunk):
  scoresT [t,s] = zkT t-tile (stationary) x zqT (moving), K=d=64, fp16
  ex = exp(scores/8), fp16: ~2/3 of tiles exact on ACT, ~1/3 on DVE via the
      Schraudolph fp16 bit-trick (one tensor_scalar: i16 = A*s + B, bitcast)
  acc matmul: stationary [vz | ones] -> rows 0:64 = unnormalized out^T,
      rows 64:128 = softmax denominator (free via ones columns)
  rec = reciprocal_approx_fast(denom) on DVE; oz = acc[0:64] * rec (fp16)

Final: yT[e, s] partial = WcT slice (stationary) x oz (moving), fp16 matmul;
  psum->sbuf fp16 copies on Pool; DMA out. Host sums 4 partials per batch.

ACT table loads: exactly 2 (Sin set for phase A, Exp set for phase B).
"""

import os
import sys

import numpy as np

if "/opt/trn_rl_repo" not in sys.path:
    sys.path.insert(0, "/opt/trn_rl_repo")

import concourse.bass as bass  # noqa: F401
import concourse.tile as tile
from concourse import bacc
from concourse import mybir
from concourse.bass_utils import run_bass_kernel_spmd

AF = mybir.ActivationFunctionType
ALU = mybir.AluOpType
F32 = mybir.dt.float32
F32R = mybir.dt.float32r
F16 = mybir.dt.float16
I16 = mybir.dt.int16

B, S, E, H, D = 2, 2048, 1024, 16, 64
NCORES = 8
HG = 4          # heads per core
EG = HG * D     # 256
P = 128
NT = S // P     # 16 s-tiles
KC = E // P     # 8 contraction tiles for the projections
HALF_PI = float(np.pi / 2)
INV_SQRT_D = 0.125  # 1/sqrt(64)

# fp16 Schraudolph: exp(s/8) ~= bitcast_f16(i16(EXP_A*s + EXP_B)); max rel
# err ~3.1% (applied to EXP_DVE of every EXP_MOD attention tiles).
EXP_A = 1024.0 * float(np.log2(np.e)) / 8.0
EXP_B = 15315.75
EXP_MOD = 3
EXP_DVE = (2,)   # which idx % EXP_MOD values go to the DVE bit-trick


def _attention(tc, psB, oz_tiles, zqT, zkT, vzts):
    nc = tc.nc
    ex_idx = 0
    with (
        tc.tile_pool(name="exps", bufs=4) as exq,
        tc.tile_pool(name="norm", bufs=2) as nrm,
    ):
        for m in range(2):
            for h2 in range(2):
                h = 2 * m + h2
                dbase = h2 * D
                oz = oz_tiles[m]
                for sb in range(4):
                    ssl = slice(sb * 512, (sb + 1) * 512)
                    acc = psB.tile([P, 512], F32, tag="acc", bufs=2,
                                   name=f"acc{h}_{sb}")
                    # software-pipelined: emit sc(t+1) before acc(t) so a
                    # stalled acc never head-of-line-blocks the next scores
                    # matmul in the PE FIFO
                    sc_tiles = []
                    for t in range(NT + 1):
                        if t < NT:
                            sc = psB.tile([P, 512], F32, tag="sc", bufs=2,
                                          name=f"sc{h}_{t}_{sb}")
                            nc.tensor.matmul(
                                sc[:],
                                lhsT=zkT[m][dbase:dbase + D, t * P:(t + 1) * P],
                                rhs=zqT[m][dbase:dbase + D, ssl],
                                start=True, stop=True,
                            )
                            ex = exq.tile([P, 512], F16, tag="ex",
                                          name=f"ex{h}_{t}_{sb}")
                            if (ex_idx % EXP_MOD) in EXP_DVE:
                                nc.vector.tensor_scalar(
                                    out=ex[:].bitcast(I16), in0=sc[:],
                                    scalar1=EXP_A, scalar2=EXP_B,
                                    op0=ALU.mult, op1=ALU.add,
                                )
                            else:
                                nc.scalar.activation(
                                    ex[:], sc[:], AF.Exp, scale=INV_SQRT_D)
                            ex_idx += 1
                            sc_tiles.append(ex)
                        if t > 0:
                            tp = t - 1
                            nc.tensor.matmul(
                                acc[:],
                                lhsT=vzts[tp][:, 2 * h:2 * h + 2, :].rearrange(
                                    "p a d -> p (a d)"),
                                rhs=sc_tiles[tp][:],
                                start=(tp == 0), stop=(tp == NT - 1),
                            )
                    rec = nrm.tile([D, 512], F32, tag="rec", name=f"rec{h}{sb}")
                    nc.vector.reciprocal_approx_fast(rec[:], acc[D:2 * D, :])
                    nc.vector.tensor_tensor(
                        out=oz[dbase:dbase + D, ssl],
                        in0=acc[0:D, :], in1=rec[:], op=ALU.mult,
                    )


def _build_body(tc, xT, wT, wcT, ident, yT):
    nc = tc.nc

    with (
        tc.tile_pool(name="const", bufs=1) as const,
        tc.tile_pool(name="wc", bufs=1) as wcp,
        tc.tile_pool(name="vz", bufs=1) as vzp,
        tc.tile_pool(name="zT", bufs=1) as zTp,
    ):
        hp = const.tile([P, 1], F32)
        nc.vector.memset(hp[:], HALF_PI)
        ones = const.tile([P, D], F16)
        nc.vector.memset(ones[:], 1.0)
        id_t = const.tile([P, P], F16)
        nc.sync.dma_start(out=id_t[:], in_=ident[:])
        wc_t = wcp.tile([P, 2, E], F16)
        nc.sync.dma_start(out=wc_t[:], in_=wcT.rearrange("(k p) e -> p k e", p=P))

        vzts = [
            vzp.tile([P, 8, D], F16, tag=f"vz{t}", name=f"vz{t}")
            for t in range(NT)
        ]
        for t in range(NT):
            nc.gpsimd.memset(vzts[t][:, 1:8:2, :], 1.0)

        zqT = [zTp.tile([P, S], F16, tag=f"zqT{m}", name=f"zqT{m}")
               for m in range(2)]
        zkT = [zTp.tile([P, S], F16, tag=f"zkT{m}", name=f"zkT{m}")
               for m in range(2)]

        # ---------------- Phase A ----------------
        with (
            tc.tile_pool(name="psA", bufs=1, space="PSUM") as psA,
            tc.tile_pool(name="psT", bufs=1, space="PSUM") as psT,
            tc.tile_pool(name="x", bufs=KC) as xp,
            tc.tile_pool(name="w", bufs=1) as wp,
            tc.tile_pool(name="c", bufs=3) as cp,
            tc.tile_pool(name="zs", bufs=1) as zsp,
        ):
            w_t = wp.tile([P, KC, 3 * EG], F16, tag="w")
            nc.sync.dma_start(
                out=w_t[:], in_=wT.rearrange("(k p) n -> p k n", p=P))
            # x arrives s-chunk-major so the first theta psum completes after
            # ~1/4 of the x bytes instead of all of them
            xts = [xp.tile([P, S], F16, tag="x", name=f"x{k}") for k in range(KC)]
            for sb in range(4):
                for k in range(KC):
                    nc.sync.dma_start(
                        out=xts[k][:, sb * 512:(sb + 1) * 512],
                        in_=xT[k * P:(k + 1) * P, sb * 512:(sb + 1) * 512],
                    )

            trans_q = []  # pipelined transposes: emit for tile t-1 during t
            for t in range(NT + 1):
                if t < NT:
                    tsl = slice(t * P, (t + 1) * P)
                    th = psA.tile([P, 3 * EG], F32, tag="th", bufs=2,
                                  name=f"th{t}")
                    for k in range(KC):
                        for nm in range(3):
                            nsl = slice(nm * EG, (nm + 1) * EG)
                            nc.tensor.matmul(
                                th[:, nsl],
                                lhsT=xts[k][:, tsl],
                                rhs=w_t[:, k, nsl],
                                start=(k == 0), stop=(k == KC - 1),
                            )
                    c = cp.tile([P, 3 * EG], F16, tag="c", name=f"c{t}")
                    nc.scalar.activation(c[:], th[:], AF.Sin, bias=hp[:])
                    zq_s = zsp.tile([P, EG], F16, tag="zq", bufs=2,
                                    name=f"zqs{t}")
                    zk_s = zsp.tile([P, EG], F16, tag="zk", bufs=2,
                                    name=f"zks{t}")
                    for h in range(HG):
                        dsl = slice(h * D, (h + 1) * D)
                        nc.vector.tensor_tensor_scan(
                            zq_s[:, dsl], c[:, dsl], ones[:], 1.0,
                            ALU.mult, ALU.bypass)
                        nc.vector.tensor_tensor_scan(
                            zk_s[:, dsl], c[:, EG + h * D:EG + (h + 1) * D],
                            ones[:], 1.0, ALU.mult, ALU.bypass)
                        nc.gpsimd.tensor_tensor_scan(
                            vzts[t][:, 2 * h, :],
                            c[:, 2 * EG + h * D:2 * EG + (h + 1) * D],
                            ones[:], 1.0, ALU.mult, ALU.bypass)
                    trans_q.append((t, zq_s, zk_s))
                if t > 0:
                    tp, zq_s, zk_s = trans_q[t - 1]
                    tsl = slice(tp * P, (tp + 1) * P)
                    for m in range(2):
                        msl = slice(m * P, (m + 1) * P)
                        for src, dst in ((zq_s, zqT[m]), (zk_s, zkT[m])):
                            pt = psT.tile([P, P], F16, tag="pt", bufs=4,
                                          name=f"pt{tp}{m}")
                            nc.tensor.transpose(pt[:], src[:, msl], id_t[:])
                            nc.gpsimd.tensor_copy(out=dst[:, tsl], in_=pt[:])

        # ---------------- Phase B + final projection ----------------
        with (
            tc.tile_pool(name="psB", bufs=1, space="PSUM") as psB,
            tc.tile_pool(name="oz", bufs=1) as ozp,
        ):
            oz_tiles = [ozp.tile([P, S], F16, tag=f"oz{m}", name=f"oz{m}")
                        for m in range(2)]
            _attention(tc, psB, oz_tiles, zqT, zkT, vzts)

            with tc.tile_pool(name="y", bufs=3) as yp:
                for mo in range(E // P):
                    for sb in range(4):
                        ssl = slice(sb * 512, (sb + 1) * 512)
                        py = psB.tile([P, 512], F32, tag="py", bufs=2,
                                      name=f"py{mo}{sb}")
                        for kk in range(2):
                            nc.tensor.matmul(
                                py[:],
                                lhsT=wc_t[:, kk, mo * P:(mo + 1) * P],
                                rhs=oz_tiles[kk][:, ssl],
                                start=(kk == 0), stop=(kk == 1),
                            )
                        yt = yp.tile([P, 512], F16, tag="y", name=f"yt{mo}{sb}")
                        nc.gpsimd.tensor_copy(out=yt[:], in_=py[:])
                        nc.sync.dma_start(
                            out=yT[mo * P:(mo + 1) * P, ssl], in_=yt[:])


def build_bass():
    nc = bacc.Bacc(None, target_bir_lowering=False)
    xT = nc.dram_tensor("xT", [E, S], F16, kind="ExternalInput")
    wT = nc.dram_tensor("wT", [E, 3 * EG], F16, kind="ExternalInput")
    wcT = nc.dram_tensor("wcT", [EG, E], F16, kind="ExternalInput")
    ident = nc.dram_tensor("ident", [P, P], F16, kind="ExternalInput")
    yT = nc.dram_tensor("yT", [E, S], F16, kind="ExternalOutput")
    with tile.TileContext(nc) as tc:
        _build_body(tc, xT[:], wT[:], wcT[:], ident[:], yT[:])
    nc.finalize()
    return nc


_NC_CACHE = None


def _get_nc():
    global _NC_CACHE
    if _NC_CACHE is None:
        _NC_CACHE = build_bass()
    return _NC_CACHE


def kernel(x, Wq, Wk, Wv, Wc, bc, **kw):
    x = np.asarray(x, np.float32)
    ident = np.eye(P, dtype=np.float16)
    in_maps = []
    for c in range(NCORES):
        b, g = divmod(c, NCORES // B)
        sl = slice(g * EG, (g + 1) * EG)
        wqkv = np.concatenate(
            [np.asarray(Wq)[sl, :].T, np.asarray(Wk)[sl, :].T,
             np.asarray(Wv)[sl, :].T], axis=1).astype(np.float16)
        in_maps.append({
            "xT": np.ascontiguousarray(np.asarray(x[b]).T.astype(np.float16)),
            "wT": np.ascontiguousarray(wqkv),
            "wcT": np.ascontiguousarray(
                np.asarray(Wc)[:, sl].T.astype(np.float16)),
            "ident": ident,
        })
    nc = _get_nc()
    res = run_bass_kernel_spmd(
        nc, in_maps, core_ids=list(range(NCORES)),
        trace=bool(int(os.environ.get("QK_TRACE", "0"))),
    )
    y = np.zeros((B, S, E), np.float32)
    for c in range(NCORES):
        b = c // (NCORES // B)
        y[b] += res.results[c]["yT"].astype(np.float32).T
    y += np.asarray(bc, np.float32)
    globals()["_LAST_RESULT"] = res
    return y


# revision 25
# speedup vs baseline: 1.5275x; 1.1864x over previous
"""Trainium2 Bass kernel: multi-head attention with quantum (cumprod-of-cos) transform.

Full-input contract: kernel(**inputs) takes the unsharded inputs and returns the
full [B, S, E] output. Internally shards over 8 NeuronCores: data-parallel over
batch (B=2) x tensor-parallel over head-groups (4 heads per core).

Per-core pipeline (b = batch, g = head-group of 4 heads, EG = 256 e-dims):

Phase A (per 128-row s-tile, 16 tiles):
  theta [s, 768] = x_tile (stationary) @ [Wq|Wk|Wv] slices (moving, fp16)
  c = sin(theta + pi/2) = cos(theta)          (one ACT op per tile, fp16 out)
  z = cumprod(c) along d (64 per head) via tensor_tensor_scan (op0=mult,
      op1=bypass): q,k scans on DVE, v scans on GPSIMD(Pool)
  zq/zk tiles [s, d] are PE-transposed ([128,128] fp16, identity-matmul) to
      zqT/zkT [d, s]; psum->sbuf copies ride Pool
  vz tiles [t, 8, 64] interleave z_v with ones columns (denominator trick)

Phase B attention (per head, per 512-wide s-chunk):
  scoresT [t,s] = zkT t-tile (stationary) x zqT (moving), K=d=64, fp16
  ex = exp(scores/8), fp16: ~2/3 of tiles exact on ACT, ~1/3 on DVE via the
      Schraudolph fp16 bit-trick (one tensor_scalar: i16 = A*s + B, bitcast)
  acc matmul: stationary [vz | ones] -> rows 0:64 = unnormalized out^T,
      rows 64:128 = softmax denominator (free via ones columns)
  rec = reciprocal_approx_fast(denom) on DVE; oz = acc[0:64] * rec (fp16)

Final: yT[e, s] partial = WcT slice (stationary) x oz (moving), fp16 matmul;
  psum->sbuf fp16 copies on Pool; DMA out. Host sums 4 partials per batch.

ACT table loads: exactly 2 (Sin set for phase A, Exp set for phase B).
"""

import os
import sys

import numpy as np

if "/opt/trn_rl_repo" not in sys.path:
    sys.path.insert(0, "/opt/trn_rl_repo")

import concourse.bass as bass  # noqa: F401
import concourse.tile as tile
from concourse import bacc
from concourse import mybir
from concourse.bass_utils import run_bass_kernel_spmd

AF = mybir.ActivationFunctionType
ALU = mybir.AluOpType
F32 = mybir.dt.float32
F32R = mybir.dt.float32r
F16 = mybir.dt.float16
I16 = mybir.dt.int16

B, S, E, H, D = 2, 2048, 1024, 16, 64
NCORES = 8
HG = 4          # heads per core
EG = HG * D     # 256
P = 128
NT = S // P     # 16 s-tiles
KC = E // P     # 8 contraction tiles for the projections
HALF_PI = float(np.pi / 2)
INV_SQRT_D = 0.125  # 1/sqrt(64)

# fp16 Schraudolph: exp(s/8) ~= bitcast_f16(i16(EXP_A*s + EXP_B)); max rel
# err ~3.1% (applied to EXP_DVE of every EXP_MOD attention tiles).
EXP_A = 1024.0 * float(np.log2(np.e)) / 8.0
EXP_B = 15315.75
EXP_MOD = 3
EXP_DVE = (2,)   # which idx % EXP_MOD values go to the DVE bit-trick


def _attention(tc, oz_tiles, zqT, zkT, vzts, dbg=None):
    nc = tc.nc
    ex_idx = 0
    # Two independent (head, s-chunk) streams are interleaved so one stream's
    # scores->exp->accumulate latency chain hides behind the other stream's
    # matmuls in the PE FIFO.
    combos = [(m, h2, sb) for m in range(2) for h2 in range(2)
              for sb in range(4)]
    with (
        tc.tile_pool(name="psB", bufs=1, space="PSUM") as psB,
        tc.tile_pool(name="exps", bufs=6) as exq,
        tc.tile_pool(name="norm", bufs=2) as nrm,
    ):
        for pi in range(0, len(combos), 2):
            pair = combos[pi:pi + 2]
            accs = []
            for s, (m, h2, sb) in enumerate(pair):
                accs.append(psB.tile([P, 512], F32, tag=f"acc{s}", bufs=2,
                                     name=f"acc{pi}_{s}"))
            exs = [[None] * NT for _ in pair]
            for t in range(NT + 1):
                for s, (m, h2, sb) in enumerate(pair):
                    h = 2 * m + h2
                    dbase = h2 * D
                    ssl = slice(sb * 512, (sb + 1) * 512)
                    if t < NT:
                        sc = psB.tile([P, 512], F32, tag=f"sc{s}", bufs=2,
                                      name=f"sc{pi}_{s}_{t}")
                        nc.tensor.matmul(
                            sc[:],
                            lhsT=zkT[m][dbase:dbase + D, t * P:(t + 1) * P],
                            rhs=zqT[m][dbase:dbase + D, ssl],
                            start=True, stop=True,
                        )
                        ex = exq.tile([P, 512], F16, tag="ex",
                                      name=f"ex{pi}_{s}_{t}")
                        if (ex_idx % EXP_MOD) in EXP_DVE:
                            nc.vector.tensor_scalar(
                                out=ex[:].bitcast(I16), in0=sc[:],
                                scalar1=EXP_A, scalar2=EXP_B,
                                op0=ALU.mult, op1=ALU.add,
                            )
                        else:
                            nc.scalar.activation(
                                ex[:], sc[:], AF.Exp, scale=INV_SQRT_D)
                        if _DEBUG and pi == 0 and s == 0 and t == 0:
                            nc.sync.dma_start(out=dbg["dbg_ex0"][:], in_=ex[:])
                        ex_idx += 1
                        exs[s][t] = ex
                    if t > 0:
                        tp = t - 1
                        nc.tensor.matmul(
                            accs[s][:],
                            lhsT=vzts[tp][:, 2 * h:2 * h + 2, :].rearrange(
                                "p a d -> p (a d)"),
                            rhs=exs[s][tp][:],
                            start=(tp == 0), stop=(tp == NT - 1),
                        )
            for s, (m, h2, sb) in enumerate(pair):
                dbase = h2 * D
                ssl = slice(sb * 512, (sb + 1) * 512)
                rec = nrm.tile([D, 512], F32, tag="rec", name=f"rec{pi}_{s}")
                nc.vector.reciprocal_approx_fast(rec[:], accs[s][0:D, :])
                if _DEBUG and pi == 0 and s == 0:
                    den = nrm.tile([D, 512], F32, tag="dbgden", name="dbgden")
                    nc.vector.tensor_copy(out=den[:], in_=accs[s][0:D, :])
                    nc.sync.dma_start(out=dbg["dbg_den0"][:], in_=den[:])
                    num = nrm.tile([D, 512], F32, tag="dbgnum", name="dbgnum")
                    nc.vector.tensor_copy(out=num[:], in_=accs[s][D:2 * D, :])
                    nc.sync.dma_start(out=dbg["dbg_num0"][:], in_=num[:])
                nc.vector.tensor_tensor(
                    out=oz_tiles[m][dbase:dbase + D, ssl],
                    in0=accs[s][D:2 * D, :], in1=rec[:], op=ALU.mult,
                )


_DEBUG = bool(int(os.environ.get("QK_DEBUG", "0")))


def _build_body(tc, xT, wT, wcT, ident, yT, dbg=None):
    nc = tc.nc

    with (
        tc.tile_pool(name="const", bufs=1) as const,
        tc.tile_pool(name="wc", bufs=1) as wcp,
        tc.tile_pool(name="vz", bufs=1) as vzp,
        tc.tile_pool(name="zT", bufs=1) as zTp,
    ):
        hp = const.tile([P, 1], F32)
        nc.vector.memset(hp[:], HALF_PI)
        ones = const.tile([P, D], F16)
        nc.vector.memset(ones[:], 1.0)
        id_t = const.tile([P, P], F16)
        nc.sync.dma_start(out=id_t[:], in_=ident[:])
        wc_t = wcp.tile([P, 2, E], F16)
        nc.sync.dma_start(out=wc_t[:], in_=wcT.rearrange("(k p) e -> p k e", p=P))

        vzts = [
            vzp.tile([P, 8, D], F16, tag=f"vz{t}", name=f"vz{t}")
            for t in range(NT)
        ]
        # slot order per head: (ones, vz) so the acc-matmul puts the softmax
        # denominator in out rows 0:64 — reciprocal_approx_fast (custom DVE
        # op) misreads PSUM inputs with nonzero base partition, so the
        # denominator must sit at partition 0
        for t in range(NT):
            nc.gpsimd.memset(vzts[t][:, 0:8:2, :], 1.0)

        zqT = [zTp.tile([P, S], F16, tag=f"zqT{m}", name=f"zqT{m}")
               for m in range(2)]
        zkT = [zTp.tile([P, S], F16, tag=f"zkT{m}", name=f"zkT{m}")
               for m in range(2)]

        # ---------------- Phase A ----------------
        with (
            tc.tile_pool(name="psA", bufs=1, space="PSUM") as psA,
            tc.tile_pool(name="psT", bufs=1, space="PSUM") as psT,
            tc.tile_pool(name="x", bufs=KC) as xp,
            tc.tile_pool(name="w", bufs=1) as wp,
            tc.tile_pool(name="c", bufs=3) as cp,
            tc.tile_pool(name="zs", bufs=1) as zsp,
        ):
            w_t = wp.tile([P, KC, 3 * EG], F16, tag="w")
            nc.sync.dma_start(
                out=w_t[:], in_=wT.rearrange("(k p) n -> p k n", p=P))
            # x arrives s-chunk-major so the first theta psum completes after
            # ~1/4 of the x bytes instead of all of them
            xts = [xp.tile([P, S], F16, tag="x", name=f"x{k}") for k in range(KC)]
            for sb in range(4):
                for k in range(KC):
                    nc.sync.dma_start(
                        out=xts[k][:, sb * 512:(sb + 1) * 512],
                        in_=xT[k * P:(k + 1) * P, sb * 512:(sb + 1) * 512],
                    )

            trans_q = []  # pipelined transposes: emit for tile t-1 during t
            for t in range(NT + 1):
                if t < NT:
                    tsl = slice(t * P, (t + 1) * P)
                    # one bank-padded psum tile per name: a matmul accumulation
                    # group's start-clear is bank-granular, so groups must not
                    # share a 2KB PSUM bank
                    ths = [psA.tile([P, EG], F32, tag=f"th{nm}", bufs=2,
                                    padded_shape=[P, 512], name=f"th{nm}_{t}")
                           for nm in range(3)]
                    for k in range(KC):
                        for nm in range(3):
                            nc.tensor.matmul(
                                ths[nm][:],
                                lhsT=xts[k][:, tsl],
                                rhs=w_t[:, k, nm * EG:(nm + 1) * EG],
                                start=(k == 0), stop=(k == KC - 1),
                            )
                    c = cp.tile([P, 3 * EG], F16, tag="c", name=f"c{t}")
                    for nm in range(3):
                        nc.scalar.activation(
                            c[:, nm * EG:(nm + 1) * EG], ths[nm][:],
                            AF.Sin, bias=hp[:])
                    zq_s = zsp.tile([P, EG], F16, tag="zq", bufs=2,
                                    name=f"zqs{t}")
                    zk_s = zsp.tile([P, EG], F16, tag="zk", bufs=2,
                                    name=f"zks{t}")
                    for h in range(HG):
                        dsl = slice(h * D, (h + 1) * D)
                        nc.vector.tensor_tensor_scan(
                            zq_s[:, dsl], c[:, dsl], ones[:], 1.0,
                            ALU.mult, ALU.bypass)
                        nc.vector.tensor_tensor_scan(
                            zk_s[:, dsl], c[:, EG + h * D:EG + (h + 1) * D],
                            ones[:], 1.0, ALU.mult, ALU.bypass)
                        nc.vector.tensor_tensor_scan(
                            vzts[t][:, 2 * h + 1, :],
                            c[:, 2 * EG + h * D:2 * EG + (h + 1) * D],
                            ones[:], 1.0, ALU.mult, ALU.bypass)
                    if _DEBUG and t == 0:
                        nc.sync.dma_start(out=dbg["dbg_c0"][:], in_=c[:])
                        nc.sync.dma_start(out=dbg["dbg_zqs0"][:], in_=zq_s[:])
                        nc.sync.dma_start(out=dbg["dbg_zks0"][:], in_=zk_s[:])
                    trans_q.append((t, zq_s, zk_s))
                if t > 0:
                    tp, zq_s, zk_s = trans_q[t - 1]
                    tsl = slice(tp * P, (tp + 1) * P)
                    for m in range(2):
                        msl = slice(m * P, (m + 1) * P)
                        for src, dst in ((zq_s, zqT[m]), (zk_s, zkT[m])):
                            pt = psT.tile([P, P], F16, tag="pt", bufs=2,
                                          padded_shape=[P, 1024],
                                          name=f"pt{tp}{m}")
                            nc.tensor.transpose(pt[:], src[:, msl], id_t[:])
                            # GPSIMD cannot read PSUM; Copy is in every ACT
                            # table so this costs no table switch
                            nc.scalar.copy(out=dst[:, tsl], in_=pt[:])

        if _DEBUG:
            nc.sync.dma_start(out=dbg["dbg_zqT0"][:], in_=zqT[0][:])
            nc.sync.dma_start(out=dbg["dbg_zkT0"][:], in_=zkT[0][:])
            nc.sync.dma_start(
                out=dbg["dbg_vz0"][:],
                in_=vzts[0][:].rearrange("p a d -> p (a d)"))

        # ---------------- Phase B + final projection ----------------
        with tc.tile_pool(name="oz", bufs=1) as ozp:
            oz_tiles = [ozp.tile([P, S], F16, tag=f"oz{m}", name=f"oz{m}")
                        for m in range(2)]
            _attention(tc, oz_tiles, zqT, zkT, vzts, dbg)
            if _DEBUG:
                nc.sync.dma_start(out=dbg["dbg_oz0"][:], in_=oz_tiles[0][:])

            with (
                tc.tile_pool(name="psY", bufs=1, space="PSUM") as psY,
                tc.tile_pool(name="y", bufs=3) as yp,
            ):
                for mo in range(E // P):
                    for sb in range(4):
                        ssl = slice(sb * 512, (sb + 1) * 512)
                        py = psY.tile([P, 512], F32, tag="py", bufs=2,
                                      name=f"py{mo}{sb}")
                        for kk in range(2):
                            nc.tensor.matmul(
                                py[:],
                                lhsT=wc_t[:, kk, mo * P:(mo + 1) * P],
                                rhs=oz_tiles[kk][:, ssl],
                                start=(kk == 0), stop=(kk == 1),
                            )
                        yt = yp.tile([P, 512], F16, tag="y", name=f"yt{mo}{sb}")
                        if (mo + sb) % 2 == 0:
                            nc.vector.tensor_copy(out=yt[:], in_=py[:])
                        else:
                            nc.scalar.copy(out=yt[:], in_=py[:])
                        nc.sync.dma_start(
                            out=yT[mo * P:(mo + 1) * P, ssl], in_=yt[:])


def build_bass():
    nc = bacc.Bacc(None, target_bir_lowering=False)
    xT = nc.dram_tensor("xT", [E, S], F16, kind="ExternalInput")
    wT = nc.dram_tensor("wT", [E, 3 * EG], F16, kind="ExternalInput")
    wcT = nc.dram_tensor("wcT", [EG, E], F16, kind="ExternalInput")
    ident = nc.dram_tensor("ident", [P, P], F16, kind="ExternalInput")
    yT = nc.dram_tensor("yT", [E, S], F16, kind="ExternalOutput")
    dbg = {}
    if _DEBUG:
        for nm, shp, dt in (("dbg_zqT0", [P, S], F16), ("dbg_zkT0", [P, S], F16),
                            ("dbg_vz0", [P, 8 * D], F16),
                            ("dbg_oz0", [P, S], F16),
                            ("dbg_c0", [P, 3 * EG], F16),
                            ("dbg_zqs0", [P, EG], F16),
                            ("dbg_zks0", [P, EG], F16),
                            ("dbg_ex0", [P, 512], F16),
                            ("dbg_den0", [D, 512], F32),
                            ("dbg_num0", [D, 512], F32)):
            dbg[nm] = nc.dram_tensor(nm, shp, dt, kind="ExternalOutput")[:]
    with tile.TileContext(nc) as tc:
        _build_body(tc, xT[:], wT[:], wcT[:], ident[:], yT[:], dbg)
    nc.finalize()
    return nc


_NC_CACHE = None


def _get_nc():
    global _NC_CACHE
    if _NC_CACHE is None:
        _NC_CACHE = build_bass()
    return _NC_CACHE


def kernel(x, Wq, Wk, Wv, Wc, bc, **kw):
    x = np.asarray(x, np.float32)
    ident = np.eye(P, dtype=np.float16)
    in_maps = []
    for c in range(NCORES):
        b, g = divmod(c, NCORES // B)
        sl = slice(g * EG, (g + 1) * EG)
        wqkv = np.concatenate(
            [np.asarray(Wq)[sl, :].T, np.asarray(Wk)[sl, :].T,
             np.asarray(Wv)[sl, :].T], axis=1).astype(np.float16)
        in_maps.append({
            "xT": np.ascontiguousarray(np.asarray(x[b]).T.astype(np.float16)),
            "wT": np.ascontiguousarray(wqkv),
            "wcT": np.ascontiguousarray(
                np.asarray(Wc)[:, sl].T.astype(np.float16)),
            "ident": ident,
        })
    nc = _get_nc()
    res = run_bass_kernel_spmd(
        nc, in_maps, core_ids=list(range(NCORES)),
        trace=bool(int(os.environ.get("QK_TRACE", "0"))),
    )
    y = np.zeros((B, S, E), np.float32)
    for c in range(NCORES):
        b = c // (NCORES // B)
        y[b] += res.results[c]["yT"].astype(np.float32).T
    y += np.asarray(bc, np.float32)
    globals()["_LAST_RESULT"] = res
    return y


# revision 30
# speedup vs baseline: 1.6543x; 1.0830x over previous
"""Trainium2 Bass kernel: multi-head attention with quantum (cumprod-of-cos) transform.

Full-input contract: kernel(**inputs) takes the unsharded inputs and returns the
full [B, S, E] output. Internally shards over 8 NeuronCores: data-parallel over
batch (B=2) x tensor-parallel over head-groups (4 heads per core).

Per-core pipeline (b = batch, g = head-group of 4 heads, EG = 256 e-dims):

Phase A (per 128-row s-tile, 16 tiles):
  theta [s, 768] = x_tile (stationary) @ [Wq|Wk|Wv] slices (moving, fp16)
  c = sin(theta + pi/2) = cos(theta)          (one ACT op per tile, fp16 out)
  z = cumprod(c) along d (64 per head) via tensor_tensor_scan (op0=mult,
      op1=bypass): q,k scans on DVE, v scans on GPSIMD(Pool)
  zq/zk tiles [s, d] are PE-transposed ([128,128] fp16, identity-matmul) to
      zqT/zkT [d, s]; psum->sbuf copies ride Pool
  vz tiles [t, 8, 64] interleave z_v with ones columns (denominator trick)

Phase B attention (per head, per 512-wide s-chunk):
  scoresT [t,s] = zkT t-tile (stationary) x zqT (moving), K=d=64, fp16
  ex = exp(scores/8), fp16: ~2/3 of tiles exact on ACT, ~1/3 on DVE via the
      Schraudolph fp16 bit-trick (one tensor_scalar: i16 = A*s + B, bitcast)
  acc matmul: stationary [vz | ones] -> rows 0:64 = unnormalized out^T,
      rows 64:128 = softmax denominator (free via ones columns)
  rec = reciprocal_approx_fast(denom) on DVE; oz = acc[0:64] * rec (fp16)

Final: yT[e, s] partial = WcT slice (stationary) x oz (moving), fp16 matmul;
  psum->sbuf fp16 copies on Pool; DMA out. Host sums 4 partials per batch.

ACT table loads: exactly 2 (Sin set for phase A, Exp set for phase B).
"""

import os
import sys

import numpy as np

if "/opt/trn_rl_repo" not in sys.path:
    sys.path.insert(0, "/opt/trn_rl_repo")

import concourse.bass as bass  # noqa: F401
import concourse.tile as tile
from concourse import bacc
from concourse import mybir
from concourse.bass_utils import run_bass_kernel_spmd

AF = mybir.ActivationFunctionType
ALU = mybir.AluOpType
F32 = mybir.dt.float32
F32R = mybir.dt.float32r
F16 = mybir.dt.float16
I16 = mybir.dt.int16

B, S, E, H, D = 2, 2048, 1024, 16, 64
NCORES = 8
HG = 4          # heads per core
EG = HG * D     # 256
P = 128
NT = S // P     # 16 s-tiles
KC = E // P     # 8 contraction tiles for the projections
HALF_PI = float(np.pi / 2)
INV_SQRT_D = 0.125  # 1/sqrt(64)

# fp16 Schraudolph: exp(s/8) ~= bitcast_f16(i16(EXP_A*s + EXP_B)); max rel
# err ~3.1% (applied to EXP_DVE of every EXP_MOD attention tiles).
EXP_A = 1024.0 * float(np.log2(np.e)) / 8.0
EXP_B = 15315.75
EXP_MOD = 3
EXP_DVE = (2,)   # which idx % EXP_MOD values go to the DVE bit-trick


def _attention(tc, oz_tiles, zqT, zkT, vzts, wc_t, yT, dbg=None):
    nc = tc.nc
    ex_idx = 0
    cp_idx = 0
    # Two independent (head, s-chunk) streams are interleaved so one stream's
    # scores->exp->accumulate latency chain hides behind the other stream's
    # matmuls in the PE FIFO. Groups run s-chunk-major so a chunk's final
    # projection can interleave into the next chunk's attention stream.
    combos = [(m, h2, sb) for sb in range(4) for m in range(2)
              for h2 in range(2)]
    pending = []  # (mo, sb) final-projection chunks whose oz slices are done

    with tc.tile_pool(name="y", bufs=3) as yp:
        def emit_final_chunk(pool, bufs):
            nonlocal cp_idx
            mo, sb = pending.pop(0)
            ssl = slice(sb * 512, (sb + 1) * 512)
            py = pool.tile([P, 512], F32, tag="py", bufs=bufs,
                           name=f"py{mo}_{sb}")
            for kk in range(2):
                nc.tensor.matmul(
                    py[:],
                    lhsT=wc_t[:, kk, mo * P:(mo + 1) * P],
                    rhs=oz_tiles[kk][:, ssl],
                    start=(kk == 0), stop=(kk == 1),
                )
            yt = yp.tile([P, 512], F16, tag="y", name=f"yt{mo}_{sb}")
            if cp_idx % 2 == 0:
                nc.vector.tensor_copy(out=yt[:], in_=py[:])
            else:
                nc.scalar.copy(out=yt[:], in_=py[:])
            cp_idx += 1
            nc.sync.dma_start(out=yT[mo * P:(mo + 1) * P, ssl], in_=yt[:])

        with (
            tc.tile_pool(name="psB", bufs=1, space="PSUM") as psB,
            tc.tile_pool(name="exps", bufs=6) as exq,
            tc.tile_pool(name="norm", bufs=2) as nrm,
        ):
            for pi in range(0, len(combos), 2):
                pair = combos[pi:pi + 2]
                accs = []
                for s, (m, h2, sb) in enumerate(pair):
                    accs.append(psB.tile([P, 512], F32, tag=f"acc{s}", bufs=2,
                                         name=f"acc{pi}_{s}"))
                exs = [[None] * NT for _ in pair]
                for t in range(NT + 1):
                    for s, (m, h2, sb) in enumerate(pair):
                        h = 2 * m + h2
                        dbase = h2 * D
                        ssl = slice(sb * 512, (sb + 1) * 512)
                        if t < NT:
                            sc = psB.tile([P, 512], F32, tag="sc", bufs=3,
                                          name=f"sc{pi}_{s}_{t}")
                            nc.tensor.matmul(
                                sc[:],
                                lhsT=zkT[m][dbase:dbase + D,
                                            t * P:(t + 1) * P],
                                rhs=zqT[m][dbase:dbase + D, ssl],
                                start=True, stop=True,
                            )
                            ex = exq.tile([P, 512], F16, tag="ex",
                                          name=f"ex{pi}_{s}_{t}")
                            if (ex_idx % EXP_MOD) in EXP_DVE:
                                nc.vector.tensor_scalar(
                                    out=ex[:].bitcast(I16), in0=sc[:],
                                    scalar1=EXP_A, scalar2=EXP_B,
                                    op0=ALU.mult, op1=ALU.add,
                                )
                            else:
                                nc.scalar.activation(
                                    ex[:], sc[:], AF.Exp, scale=INV_SQRT_D)
                            if _DEBUG and pi == 0 and s == 0 and t == 0:
                                nc.sync.dma_start(
                                    out=dbg["dbg_ex0"][:], in_=ex[:])
                            ex_idx += 1
                            exs[s][t] = ex
                        if t > 0:
                            tp = t - 1
                            nc.tensor.matmul(
                                accs[s][:],
                                lhsT=vzts[tp][:, 2 * h:2 * h + 2, :].rearrange(
                                    "p a d -> p (a d)"),
                                rhs=exs[s][tp][:],
                                start=(tp == 0), stop=(tp == NT - 1),
                            )
                    # at most one pending final-projection chunk per t slot:
                    # the interleave gives each py's copy a full iteration to
                    # drain, so py bufs=1 never head-of-line-blocks the PE FIFO
                    if pending and t < NT:
                        emit_final_chunk(psB, 1)
                for s, (m, h2, sb) in enumerate(pair):
                    dbase = h2 * D
                    ssl = slice(sb * 512, (sb + 1) * 512)
                    rec = nrm.tile([D, 512], F32, tag="rec",
                                   name=f"rec{pi}_{s}")
                    nc.vector.reciprocal_approx_fast(rec[:], accs[s][0:D, :])
                    if _DEBUG and pi == 0 and s == 0:
                        den = nrm.tile([D, 512], F32, tag="dbgden",
                                       name="dbgden")
                        nc.vector.tensor_copy(out=den[:], in_=accs[s][0:D, :])
                        nc.sync.dma_start(out=dbg["dbg_den0"][:], in_=den[:])
                        num = nrm.tile([D, 512], F32, tag="dbgnum",
                                       name="dbgnum")
                        nc.vector.tensor_copy(out=num[:],
                                              in_=accs[s][D:2 * D, :])
                        nc.sync.dma_start(out=dbg["dbg_num0"][:], in_=num[:])
                    nc.vector.tensor_tensor(
                        out=oz_tiles[m][dbase:dbase + D, ssl],
                        in0=accs[s][D:2 * D, :], in1=rec[:], op=ALU.mult,
                    )
                if pair[-1][0] == 1:  # both m-pairs of this sb done
                    sb = pair[-1][2]
                    pending.extend((mo, sb) for mo in range(E // P))

        # drain the remaining chunks (last s-chunk) in a fresh psum pool with
        # deeper buffering, now that the attention banks are free
        with tc.tile_pool(name="psF", bufs=1, space="PSUM") as psF:
            while pending:
                emit_final_chunk(psF, 3)


_DEBUG = bool(int(os.environ.get("QK_DEBUG", "0")))


def _build_body(tc, xT, wT, wcT, ident, yT, dbg=None):
    nc = tc.nc

    with (
        tc.tile_pool(name="const", bufs=1) as const,
        tc.tile_pool(name="wc", bufs=1) as wcp,
        tc.tile_pool(name="vz", bufs=1) as vzp,
        tc.tile_pool(name="zT", bufs=1) as zTp,
    ):
        hp = const.tile([P, 1], F32)
        nc.vector.memset(hp[:], HALF_PI)
        ones = const.tile([P, D], F16)
        nc.vector.memset(ones[:], 1.0)
        id_t = const.tile([P, P], F16)
        nc.sync.dma_start(out=id_t[:], in_=ident[:])
        wc_t = wcp.tile([P, 2, E], F16)
        nc.sync.dma_start(out=wc_t[:], in_=wcT.rearrange("(k p) e -> p k e", p=P))

        vzts = [
            vzp.tile([P, 8, D], F16, tag=f"vz{t}", name=f"vz{t}")
            for t in range(NT)
        ]
        # slot order per head: (ones, vz) so the acc-matmul puts the softmax
        # denominator in out rows 0:64 — reciprocal_approx_fast (custom DVE
        # op) misreads PSUM inputs with nonzero base partition, so the
        # denominator must sit at partition 0
        for t in range(NT):
            nc.gpsimd.memset(vzts[t][:, 0:8:2, :], 1.0)

        zqT = [zTp.tile([P, S], F16, tag=f"zqT{m}", name=f"zqT{m}")
               for m in range(2)]
        zkT = [zTp.tile([P, S], F16, tag=f"zkT{m}", name=f"zkT{m}")
               for m in range(2)]

        # ---------------- Phase A ----------------
        with (
            tc.tile_pool(name="psA", bufs=1, space="PSUM") as psA,
            tc.tile_pool(name="psT", bufs=1, space="PSUM") as psT,
            tc.tile_pool(name="x", bufs=KC) as xp,
            tc.tile_pool(name="w", bufs=1) as wp,
            tc.tile_pool(name="c", bufs=3) as cp,
            tc.tile_pool(name="zs", bufs=1) as zsp,
        ):
            # x arrives s-chunk-major so the first theta psum completes after
            # ~1/4 of the x bytes instead of all of them; w arrives per
            # k-chunk interleaved with the first x chunk so matmul k=0 can
            # start after ~0.5 MB of DMA
            w_t = wp.tile([P, KC, 3 * EG], F16, tag="w")
            wTr = wT.rearrange("(k p) n -> p k n", p=P)
            xts = [xp.tile([P, S], F16, tag="x", name=f"x{k}") for k in range(KC)]
            for sb in range(4):
                for k in range(KC):
                    if sb == 0:
                        nc.sync.dma_start(out=w_t[:, k, :], in_=wTr[:, k, :])
                    nc.sync.dma_start(
                        out=xts[k][:, sb * 512:(sb + 1) * 512],
                        in_=xT[k * P:(k + 1) * P, sb * 512:(sb + 1) * 512],
                    )

            trans_q = []  # pipelined transposes: emit for tile t-1 during t
            for t in range(NT + 1):
                if t < NT:
                    tsl = slice(t * P, (t + 1) * P)
                    # one bank-padded psum tile per name: a matmul accumulation
                    # group's start-clear is bank-granular, so groups must not
                    # share a 2KB PSUM bank
                    ths = [psA.tile([P, EG], F32, tag=f"th{nm}", bufs=2,
                                    padded_shape=[P, 512], name=f"th{nm}_{t}")
                           for nm in range(3)]
                    for k in range(KC):
                        for nm in range(3):
                            nc.tensor.matmul(
                                ths[nm][:],
                                lhsT=xts[k][:, tsl],
                                rhs=w_t[:, k, nm * EG:(nm + 1) * EG],
                                start=(k == 0), stop=(k == KC - 1),
                            )
                    c = cp.tile([P, 3 * EG], F16, tag="c", name=f"c{t}")
                    for nm in range(3):
                        nc.scalar.activation(
                            c[:, nm * EG:(nm + 1) * EG], ths[nm][:],
                            AF.Sin, bias=hp[:])
                    zq_s = zsp.tile([P, EG], F16, tag="zq", bufs=2,
                                    name=f"zqs{t}")
                    zk_s = zsp.tile([P, EG], F16, tag="zk", bufs=2,
                                    name=f"zks{t}")
                    for h in range(HG):
                        dsl = slice(h * D, (h + 1) * D)
                        nc.vector.tensor_tensor_scan(
                            zq_s[:, dsl], c[:, dsl], ones[:], 1.0,
                            ALU.mult, ALU.bypass)
                        nc.vector.tensor_tensor_scan(
                            zk_s[:, dsl], c[:, EG + h * D:EG + (h + 1) * D],
                            ones[:], 1.0, ALU.mult, ALU.bypass)
                        nc.vector.tensor_tensor_scan(
                            vzts[t][:, 2 * h + 1, :],
                            c[:, 2 * EG + h * D:2 * EG + (h + 1) * D],
                            ones[:], 1.0, ALU.mult, ALU.bypass)
                    if _DEBUG and t == 0:
                        nc.sync.dma_start(out=dbg["dbg_c0"][:], in_=c[:])
                        nc.sync.dma_start(out=dbg["dbg_zqs0"][:], in_=zq_s[:])
                        nc.sync.dma_start(out=dbg["dbg_zks0"][:], in_=zk_s[:])
                    trans_q.append((t, zq_s, zk_s))
                if t > 0:
                    tp, zq_s, zk_s = trans_q[t - 1]
                    tsl = slice(tp * P, (tp + 1) * P)
                    for m in range(2):
                        msl = slice(m * P, (m + 1) * P)
                        for src, dst in ((zq_s, zqT[m]), (zk_s, zkT[m])):
                            pt = psT.tile([P, P], F16, tag="pt", bufs=2,
                                          padded_shape=[P, 1024],
                                          name=f"pt{tp}{m}")
                            nc.tensor.transpose(pt[:], src[:, msl], id_t[:])
                            # GPSIMD cannot read PSUM; Copy is in every ACT
                            # table so this costs no table switch
                            nc.scalar.copy(out=dst[:, tsl], in_=pt[:])

        if _DEBUG:
            nc.sync.dma_start(out=dbg["dbg_zqT0"][:], in_=zqT[0][:])
            nc.sync.dma_start(out=dbg["dbg_zkT0"][:], in_=zkT[0][:])
            nc.sync.dma_start(
                out=dbg["dbg_vz0"][:],
                in_=vzts[0][:].rearrange("p a d -> p (a d)"))

        # ---------------- Phase B (attention + fused final projection) ----
        with tc.tile_pool(name="oz", bufs=1) as ozp:
            oz_tiles = [ozp.tile([P, S], F16, tag=f"oz{m}", name=f"oz{m}")
                        for m in range(2)]
            _attention(tc, oz_tiles, zqT, zkT, vzts, wc_t, yT, dbg)
            if _DEBUG:
                nc.sync.dma_start(out=dbg["dbg_oz0"][:], in_=oz_tiles[0][:])


def build_bass():
    nc = bacc.Bacc(None, target_bir_lowering=False)
    xT = nc.dram_tensor("xT", [E, S], F16, kind="ExternalInput")
    wT = nc.dram_tensor("wT", [E, 3 * EG], F16, kind="ExternalInput")
    wcT = nc.dram_tensor("wcT", [EG, E], F16, kind="ExternalInput")
    ident = nc.dram_tensor("ident", [P, P], F16, kind="ExternalInput")
    yT = nc.dram_tensor("yT", [E, S], F16, kind="ExternalOutput")
    dbg = {}
    if _DEBUG:
        for nm, shp, dt in (("dbg_zqT0", [P, S], F16), ("dbg_zkT0", [P, S], F16),
                            ("dbg_vz0", [P, 8 * D], F16),
                            ("dbg_oz0", [P, S], F16),
                            ("dbg_c0", [P, 3 * EG], F16),
                            ("dbg_zqs0", [P, EG], F16),
                            ("dbg_zks0", [P, EG], F16),
                            ("dbg_ex0", [P, 512], F16),
                            ("dbg_den0", [D, 512], F32),
                            ("dbg_num0", [D, 512], F32)):
            dbg[nm] = nc.dram_tensor(nm, shp, dt, kind="ExternalOutput")[:]
    with tile.TileContext(nc) as tc:
        _build_body(tc, xT[:], wT[:], wcT[:], ident[:], yT[:], dbg)
    nc.finalize()
    return nc


_NC_CACHE = None


def _get_nc():
    global _NC_CACHE
    if _NC_CACHE is None:
        _NC_CACHE = build_bass()
    return _NC_CACHE


def kernel(x, Wq, Wk, Wv, Wc, bc, **kw):
    x = np.asarray(x, np.float32)
    ident = np.eye(P, dtype=np.float16)
    in_maps = []
    for c in range(NCORES):
        b, g = divmod(c, NCORES // B)
        sl = slice(g * EG, (g + 1) * EG)
        wqkv = np.concatenate(
            [np.asarray(Wq)[sl, :].T, np.asarray(Wk)[sl, :].T,
             np.asarray(Wv)[sl, :].T], axis=1).astype(np.float16)
        in_maps.append({
            "xT": np.ascontiguousarray(np.asarray(x[b]).T.astype(np.float16)),
            "wT": np.ascontiguousarray(wqkv),
            "wcT": np.ascontiguousarray(
                np.asarray(Wc)[:, sl].T.astype(np.float16)),
            "ident": ident,
        })
    nc = _get_nc()
    res = run_bass_kernel_spmd(
        nc, in_maps, core_ids=list(range(NCORES)),
        trace=bool(int(os.environ.get("QK_TRACE", "0"))),
    )
    y = np.zeros((B, S, E), np.float32)
    for c in range(NCORES):
        b = c // (NCORES // B)
        y[b] += res.results[c]["yT"].astype(np.float32).T
    y += np.asarray(bc, np.float32)
    globals()["_LAST_RESULT"] = res
    return y
